# revision 8
# baseline (speedup 1.0000x reference)
"""Masked dot-product attention (B=64, Lq=Lk=1024, d=64, fp32) on 8 TRN2 cores.

v2 strategy (per core: 8 batch slots, ragged k-tiles, sorted+dealt):
  - All inputs bf16. Host folds 1/sqrt(d) into Q. Masking is NOT in the
    score matmul: dead k rows (k >= valid_len) are zeroed in V (including
    the ones-column that produces softmax denominators), so whatever the
    exp stage emits for dead scores is multiplied by zero in the O matmul.
  - S^T[k,q] per k-tile via bf16 matmul (contraction d=64), PSUM f32.
  - exp is split across TWO engines to break the single-engine exp wall:
      ACT: exact exp (PSUM->SBUF bf16)
      DVE: Schraudolph fast-exp: i16 = rint(S*(2^7/ln2) + 127*2^7), whose
           bit pattern IS bf16(exp(S)) to ~3%; f32->i16 convert saturates
           (verified on HW) so dead scores (~-1e6) become 0x8000 = -0.0.
           The +3%-band bias cancels in the softmax division; using the
           uncorrected constant keeps exp(0)=1.0 exactly so valid_len==0
           batches (host zeroes Q) stay exactly uniform.
    Small batches (<=2 k-tiles) are ACT-only: Schraudolph error hurts most
    when few keys are live.
  - O^T[q,j] = sum_k P^T[k,q-chunk]^T V[k,j]: lhsT = P^T chunk [128,128],
    rhs = V-tile [128,65] (64 dims + ones column) -> out [128q, 65], only
    65 PE rows per matmul (vs 1024 streaming V^T P). PSUM accumulation
    groups clear has_written bank-wide on start, so the 8 q-chunks run as
    2 passes x 4 chunks, each chunk in its own PSUM bank ([128,4,512] f32
    tile, single buffer); pass1 re-reads the kept P tiles. O-work is a
    global FIFO drained between tiles so the PE stream never blocks on a
    PSUM buffer freed by later instructions.
  - copies PSUM->SBUF (engine chosen by load balance), output DMAs issued
    from GpSimd (SWDGE) keeping SP.SEQ/HWDGE for inputs only.
"""

import math
from collections import deque

import numpy as np
import ml_dtypes

import concourse.bass as bass
import concourse.mybir as mybir
import concourse.tile as tile
from concourse import bacc
from concourse.bass_utils import run_bass_kernel_spmd

N_CORES = 8
B = 64
L = 1024
D = 64
BPC = B // N_CORES
KT = L // 128

F32 = mybir.dt.float32
BF16 = mybir.dt.bfloat16
I16 = mybir.dt.int16
BF16NP = ml_dtypes.bfloat16

A16 = 128.0 / math.log(2.0)   # 184.6617
B16 = 127.0 * 128.0           # 16256.0

ACT_EXP_NS = 1038.0
DVE_EXP_NS = 1191.0
ACT_CP_NS = 293.0
DVE_CP_NS = 260.0

_prog_cache = {}


def _plan(ns):
    """Execution order, per-tile engine map, per-copy engine map."""
    # head: a small ACT-only batch; tail: the smallest batch (short drain).
    exec_order = ([BPC - 2] + list(range(BPC - 2)) + [BPC - 1])
    tiles = [(b, kt) for b in exec_order for kt in range(ns[b])]
    busy = {"A": 0.0, "D": 0.0}
    eng = {}
    for (b, kt) in tiles:
        if ns[b] <= 2:
            e = "A"   # accuracy: few live keys -> exact exp
        elif b == 0 and kt < 2:
            e = "D"   # wake DVE early in the head
        elif busy["A"] + ACT_EXP_NS <= busy["D"] + DVE_EXP_NS:
            e = "A"
        else:
            e = "D"
        eng[(b, kt)] = e
        busy[e] += ACT_EXP_NS if e == "A" else DVE_EXP_NS
    cpeng = {}
    for b in exec_order:
        for p in range(4):
            if busy["A"] + ACT_CP_NS <= busy["D"] + DVE_CP_NS:
                cpeng[(b, p)] = "A"
                busy["A"] += ACT_CP_NS
            else:
                cpeng[(b, p)] = "D"
                busy["D"] += DVE_CP_NS
    return exec_order, tiles, eng, cpeng


def _build_program(ns):
    """ns: per-slot k-tile counts (tuple of BPC ints in 1..KT)."""
    nc = bacc.Bacc("TRN2", target_bir_lowering=False, debug=False,
                   num_devices=N_CORES)
    exec_order, tiles, eng, cpeng = _plan(ns)

    # qkt: [ktile0 (128) | qt (1024) | ktile1.. (896)] bf16, 64 partitions
    qkt_d = nc.dram_tensor("qkt", [BPC, D, 2 * L + 128], BF16,
                           kind="ExternalInput")
    vp_d = nc.dram_tensor("vp", [BPC, 128, KT, D + 1], BF16,
                          kind="ExternalInput")
    o_d = nc.dram_tensor("o", [BPC, 128, KT, D + 1], F32,
                         kind="ExternalOutput")

    with tile.TileContext(nc) as tc:
        with (
            tc.tile_pool(name="qk", bufs=1) as qk_pool,
            tc.tile_pool(name="vpp", bufs=1) as vp_pool,
            tc.tile_pool(name="pt", bufs=12) as pt_pool,
            tc.tile_pool(name="osb", bufs=3) as osb_pool,
            tc.tile_pool(name="sp", bufs=3, space="PSUM") as sp_pool,
            tc.tile_pool(name="op", bufs=1, space="PSUM") as op_pool,
        ):
            qkt_s = {}
            vp_s = {}
            first = exec_order[0]
            for b in exec_order:
                nkt = ns[b]
                end = 128 + L + (nkt - 1) * 128
                q_t = qk_pool.tile([D, 2 * L + 128], BF16, tag=f"qkt{b}")
                v_t = vp_pool.tile([128, KT, D + 1], BF16, tag=f"vp{b}")
                if b == first:
                    nc.sync.dma_start(q_t[:, :640], qkt_d[b][:, :640])
                    nc.sync.dma_start(v_t[:, :nkt, :], vp_d[b][:, :nkt, :])
                    if end > 640:
                        nc.sync.dma_start(q_t[:, 640:end], qkt_d[b][:, 640:end])
                else:
                    nc.sync.dma_start(q_t[:, :end], qkt_d[b][:, :end])
                    nc.sync.dma_start(v_t[:, :nkt, :], vp_d[b][:, :nkt, :])
                qkt_s[b] = q_t
                vp_s[b] = v_t

            def ktm_sl(b, kt):
                if kt == 0:
                    return qkt_s[b][:, :128]
                o = 128 + L + (kt - 1) * 128
                return qkt_s[b][:, o:o + 128]

            def qt_sl(b):
                return qkt_s[b][:, 128:128 + L]

            state = {}  # b -> dict(op0/op1 tiles, osb, pt list)
            owork = deque()

            def emit_exp(b, kt, sp, pt, split):
                e = eng[(b, kt)]
                if e == "A":
                    if split:
                        for h in range(2):
                            sl = slice(h * 512, (h + 1) * 512)
                            nc.scalar.activation(
                                pt[:, sl].bitcast(BF16), sp[:, sl],
                                mybir.ActivationFunctionType.Exp)
                    else:
                        nc.scalar.activation(
                            pt[:].bitcast(BF16), sp[:],
                            mybir.ActivationFunctionType.Exp)
                else:
                    nc.vector.tensor_scalar(
                        pt[:], sp[:], A16, B16,
                        mybir.AluOpType.mult, mybir.AluOpType.add)

            last_b = exec_order[-1]

            def emit_copy(b, p, op_t):
                e = cpeng[(b, p)]
                st = state[b]
                dst = st["osb"][:, 2 * p:2 * p + 2, :]
                src = op_t[:, :, :D + 1]
                if e == "A":
                    nc.scalar.copy(dst, src)
                else:
                    nc.vector.tensor_copy(dst, src)
                if p % 2 == 1:
                    # one output DMA per osb half; the final batch issues
                    # from SP (HWDGE, shorter issue path) for a short tail
                    h = p // 2
                    dsl = slice(4 * h, 4 * h + 4)
                    dma_eng = nc.sync if b == last_b else nc.gpsimd
                    dma_eng.dma_start(o_d[b][:, dsl, :], st["osb"][:, dsl, :])

            def o_unit_pass0(b, kt):
                def f():
                    st = state[b]
                    if kt == 0:
                        st["op0"] = op_pool.tile(
                            [128, 2, 512], F32, tag="op", name=f"op0_{b}")
                    op_t = st["op0"]
                    nkt = ns[b]
                    ptile = st["pt"][kt]
                    for c in range(2):
                        nc.tensor.matmul(
                            op_t[:, c, :D + 1],
                            ptile[:, c * 128:(c + 1) * 128].bitcast(BF16),
                            vp_s[b][:, kt, :],
                            start=(kt == 0), stop=(kt == nkt - 1))
                    if kt == nkt - 1:
                        emit_copy(b, 0, op_t)
                return f

            def o_unit_pass(b, p):
                def f():
                    st = state[b]
                    nkt = ns[b]
                    op_t = op_pool.tile([128, 2, 512], F32, tag="op",
                                        name=f"op{p}_{b}")
                    for kt in range(nkt):
                        ptile = st["pt"][kt]
                        for c in range(2):
                            cc = 2 * p + c
                            nc.tensor.matmul(
                                op_t[:, c, :D + 1],
                                ptile[:, cc * 128:(cc + 1) * 128]
                                .bitcast(BF16),
                                vp_s[b][:, kt, :],
                                start=(kt == 0), stop=(kt == nkt - 1))
                    emit_copy(b, p, op_t)
                return f

            for i, (b, kt) in enumerate(tiles):
                nkt = ns[b]
                if kt == 0:
                    state[b] = {
                        "pt": [],
                        "osb": osb_pool.tile([128, KT, D + 1], F32,
                                             tag="osb", name=f"osb{b}"),
                    }
                sp = sp_pool.tile([128, L], F32, tag="sp")
                for h in range(2):
                    sl = slice(h * 512, (h + 1) * 512)
                    nc.tensor.matmul(sp[:, sl], ktm_sl(b, kt),
                                     qt_sl(b)[:, sl], start=True, stop=True)
                pt = pt_pool.tile([128, L], I16, tag="pt")
                state[b]["pt"].append(pt)
                emit_exp(b, kt, sp, pt, split=(i == 0))
                owork.append(o_unit_pass0(b, kt))
                if kt == nkt - 1:
                    for p in range(1, 4):
                        owork.append(o_unit_pass(b, p))
                # drain O-work lagging the exp stream so the in-order PE
                # queue never waits on an exp that is still far out, while
                # the single op buffer hand-off stays ahead of its readers.
                while len(owork) > 3:
                    owork.popleft()()
            while owork:
                owork.popleft()()

    nc.compile()
    return nc


def get_program(ns):
    ns = tuple(ns)
    if ns not in _prog_cache:
        _prog_cache[ns] = _build_program(ns)
    return _prog_cache[ns]


def _prep_inputs(q, k, v, vl):
    """q,k,v: [n, L, D] f32; vl: [n] int. Returns (qkt, vp) bf16 arrays."""
    n = q.shape[0]
    qkt = np.zeros((n, D, 2 * L + 128), BF16NP)
    qt = (q.transpose(0, 2, 1) * np.float32(1.0 / np.sqrt(D))).astype(BF16NP)
    zmask = vl == 0
    if zmask.any():
        qt[zmask] = 0
    ktm = k.transpose(0, 2, 1).astype(BF16NP)
    qkt[:, :, :128] = ktm[:, :, :128]
    qkt[:, :, 128:128 + L] = qt
    qkt[:, :, 128 + L:2 * L] = ktm[:, :, 128:]
    vp = np.empty((n, L, D + 1), np.float32)
    vp[:, :, :D] = v
    vp[:, :, D] = 1.0
    iota = np.arange(L)
    dead = (iota[None, :] >= vl[:, None]) & ~zmask[:, None]
    vp[dead] = 0.0
    vp = vp.astype(BF16NP)
    vp = np.ascontiguousarray(
        vp.reshape(n, KT, 128, D + 1).transpose(0, 2, 1, 3))
    return qkt, vp


def kernel(queries, keys, values, valid_lens):
    queries = np.asarray(queries, np.float32)
    keys = np.asarray(keys, np.float32)
    values = np.asarray(values, np.float32)
    vl = np.asarray(valid_lens).astype(np.int64)

    # Ragged load balancing: sort batches by active k-tile count descending,
    # deal across cores; slot s runs max-of-group tiles on every core.
    nact = np.where(vl == 0, KT, -(-vl // 128)).astype(np.int64)
    order = np.argsort(-nact, kind="stable")
    ns = tuple(int(nact[order[s * N_CORES]]) for s in range(BPC))

    qkt, vp = _prep_inputs(queries[order], keys[order], values[order],
                           vl[order])

    nc = get_program(ns)
    in_maps = []
    for c in range(N_CORES):
        idx = [s * N_CORES + c for s in range(BPC)]
        in_maps.append({
            "qkt": np.ascontiguousarray(qkt[idx]),
            "vp": np.ascontiguousarray(vp[idx]),
        })

    res = None
    for attempt in range(3):
        try:
            res = run_bass_kernel_spmd(nc, in_maps, list(range(N_CORES)))
            break
        except Exception:
            if attempt == 2:
                raise
            import time as _time
            _time.sleep(2.0)
            try:
                import jax
                jax.clear_caches()
            except Exception:
                pass

    out = np.empty((B, L, D), np.float32)
    for c in range(N_CORES):
        o = res.results[c]["o"]  # [BPC, 128, KT, D+1]
        o = np.asarray(o, np.float32).transpose(0, 2, 1, 3).reshape(
            BPC, L, D + 1)
        on = o[:, :, :D] / o[:, :, D:D + 1]
        for s in range(BPC):
            out[order[s * N_CORES + c]] = on[s]
    return out


# revision 11
# speedup vs baseline: 1.2267x; 1.2267x over previous
"""Masked dot-product attention (B=64, Lq=Lk=1024, d=64, fp32) on 8 TRN2 cores.

v2 strategy (per core: 8 batch slots, ragged k-tiles, sorted+dealt):
  - All inputs bf16. Host folds 1/sqrt(d) into Q. Masking is NOT in the
    score matmul: dead k rows (k >= valid_len) are zeroed in V (including
    the ones-column that produces softmax denominators), so whatever the
    exp stage emits for dead scores is multiplied by zero in the O matmul.
  - S^T[k,q] per k-tile via bf16 matmul (contraction d=64), PSUM f32.
  - exp is split across TWO engines to break the single-engine exp wall:
      ACT: exact exp (PSUM->SBUF bf16)
      DVE: Schraudolph fast-exp: i16 = rint(S*(2^7/ln2) + 127*2^7), whose
           bit pattern IS bf16(exp(S)) to ~3%; f32->i16 convert saturates
           (verified on HW) so dead scores (~-1e6) become 0x8000 = -0.0.
           The +3%-band bias cancels in the softmax division; using the
           uncorrected constant keeps exp(0)=1.0 exactly so valid_len==0
           batches (host zeroes Q) stay exactly uniform.
    Small batches (<=2 k-tiles) are ACT-only: Schraudolph error hurts most
    when few keys are live.
  - O^T[q,j] = sum_k P^T[k,q-chunk]^T V[k,j]: lhsT = P^T chunk [128,128],
    rhs = V-tile [128,65] (64 dims + ones column) -> out [128q, 65], only
    65 PE rows per matmul (vs 1024 streaming V^T P). PSUM accumulation
    groups clear has_written bank-wide on start, so the 8 q-chunks run as
    2 passes x 4 chunks, each chunk in its own PSUM bank ([128,4,512] f32
    tile, single buffer); pass1 re-reads the kept P tiles. O-work is a
    global FIFO drained between tiles so the PE stream never blocks on a
    PSUM buffer freed by later instructions.
  - copies PSUM->SBUF (engine chosen by load balance), output DMAs issued
    from GpSimd (SWDGE) keeping SP.SEQ/HWDGE for inputs only.
"""

import math
from collections import deque

import numpy as np
import ml_dtypes

import concourse.bass as bass
import concourse.mybir as mybir
import concourse.tile as tile
from concourse import bacc
from concourse.bass_utils import run_bass_kernel_spmd

N_CORES = 8
B = 64
L = 1024
D = 64
BPC = B // N_CORES
KT = L // 128

F32 = mybir.dt.float32
BF16 = mybir.dt.bfloat16
I16 = mybir.dt.int16
BF16NP = ml_dtypes.bfloat16

A16 = 128.0 / math.log(2.0)   # 184.6617
B16 = 127.0 * 128.0           # 16256.0

ACT_EXP_NS = 1038.0
DVE_EXP_NS = 1191.0
ACT_CP_NS = 402.0
DVE_CP_NS = 396.0

_prog_cache = {}


def _plan(ns):
    """Execution order, per-tile engine map, per-copy engine map."""
    # head: a small ACT-only batch; tail: the smallest batch (short drain).
    exec_order = ([BPC - 2] + list(range(BPC - 2)) + [BPC - 1])
    tiles = [(b, kt) for b in exec_order for kt in range(ns[b])]
    busy = {"A": 0.0, "D": 0.0}
    eng = {}
    for (b, kt) in tiles:
        if ns[b] <= 2:
            e = "A"   # accuracy: few live keys -> exact exp
        elif b == 0 and kt < 2:
            e = "D"   # wake DVE early in the head
        elif busy["A"] + ACT_EXP_NS <= busy["D"] + DVE_EXP_NS:
            e = "A"
        else:
            e = "D"
        eng[(b, kt)] = e
        busy[e] += ACT_EXP_NS if e == "A" else DVE_EXP_NS
    cpeng = {}
    for b in exec_order:
        for p in range(2):
            if busy["A"] + ACT_CP_NS <= busy["D"] + DVE_CP_NS:
                cpeng[(b, p)] = "A"
                busy["A"] += ACT_CP_NS
            else:
                cpeng[(b, p)] = "D"
                busy["D"] += DVE_CP_NS
    return exec_order, tiles, eng, cpeng


def _build_program(ns):
    """ns: per-slot k-tile counts (tuple of BPC ints in 1..KT)."""
    nc = bacc.Bacc("TRN2", target_bir_lowering=False, debug=False,
                   num_devices=N_CORES)
    exec_order, tiles, eng, cpeng = _plan(ns)

    # qkt: [ktile0 (128) | qt (1024) | ktile1.. (896)] bf16, 64 partitions
    qkt_d = nc.dram_tensor("qkt", [BPC, D, 2 * L + 128], BF16,
                           kind="ExternalInput")
    vp_d = nc.dram_tensor("vp", [BPC, 128, KT, D + 1], BF16,
                          kind="ExternalInput")
    o_d = nc.dram_tensor("o", [BPC, 128, 2, 4, D + 1], F32,
                         kind="ExternalOutput")

    with tile.TileContext(nc) as tc:
        with (
            tc.tile_pool(name="qk", bufs=1) as qk_pool,
            tc.tile_pool(name="vpp", bufs=1) as vp_pool,
            tc.tile_pool(name="pt", bufs=12) as pt_pool,
            tc.tile_pool(name="osb", bufs=3) as osb_pool,
            tc.tile_pool(name="sp", bufs=3, space="PSUM") as sp_pool,
            tc.tile_pool(name="op", bufs=1, space="PSUM") as op_pool,
        ):
            qkt_s = {}
            vp_s = {}
            first = exec_order[0]
            for b in exec_order:
                nkt = ns[b]
                end = 128 + L + (nkt - 1) * 128
                q_t = qk_pool.tile([D, 2 * L + 128], BF16, tag=f"qkt{b}")
                v_t = vp_pool.tile([128, KT, D + 1], BF16, tag=f"vp{b}")
                if b == first:
                    nc.sync.dma_start(q_t[:, :640], qkt_d[b][:, :640])
                    nc.sync.dma_start(v_t[:, :nkt, :], vp_d[b][:, :nkt, :])
                    if end > 640:
                        nc.sync.dma_start(q_t[:, 640:end], qkt_d[b][:, 640:end])
                else:
                    nc.sync.dma_start(q_t[:, :end], qkt_d[b][:, :end])
                    nc.sync.dma_start(v_t[:, :nkt, :], vp_d[b][:, :nkt, :])
                qkt_s[b] = q_t
                vp_s[b] = v_t

            def ktm_sl(b, kt):
                if kt == 0:
                    return qkt_s[b][:, :128]
                o = 128 + L + (kt - 1) * 128
                return qkt_s[b][:, o:o + 128]

            def qt_sl(b):
                return qkt_s[b][:, 128:128 + L]

            state = {}  # b -> dict(op0/op1 tiles, osb, pt list)
            owork = deque()

            def emit_exp(b, kt, sp, pt, split):
                e = eng[(b, kt)]
                if e == "A":
                    if split:
                        for h in range(2):
                            sl = slice(h * 512, (h + 1) * 512)
                            nc.scalar.activation(
                                pt[:, sl].bitcast(BF16), sp[:, sl],
                                mybir.ActivationFunctionType.Exp)
                    else:
                        nc.scalar.activation(
                            pt[:].bitcast(BF16), sp[:],
                            mybir.ActivationFunctionType.Exp)
                else:
                    nc.vector.tensor_scalar(
                        pt[:], sp[:], A16, B16,
                        mybir.AluOpType.mult, mybir.AluOpType.add)

            last_b = exec_order[-1]

            def o_unit_bank(b, bk):
                # Deferred O-phase for one PSUM bank: 4 q-chunks as
                # SEQUENTIAL accumulation groups (chunk-major), legal in a
                # shared bank because no group starts while another in the
                # same bank is still accumulating.
                def f():
                    st = state[b]
                    nkt = ns[b]
                    if bk == 0:
                        st["op"] = op_pool.tile([128, 2, 4, 128], F32,
                                                tag="op", name=f"op_{b}")
                    op_t = st["op"]
                    for c in range(4):
                        for kt in range(nkt):
                            cc = 4 * bk + c
                            nc.tensor.matmul(
                                op_t[:, bk, c, :D + 1],
                                st["pt"][kt][:, cc * 128:(cc + 1) * 128]
                                .bitcast(BF16),
                                vp_s[b][:, kt, :],
                                start=(kt == 0), stop=(kt == nkt - 1))
                    e = cpeng[(b, bk)]
                    dst = st["osb"][:, bk, :, :]
                    src = op_t[:, bk, :, :D + 1]
                    if e == "A":
                        nc.scalar.copy(dst, src)
                    else:
                        nc.vector.tensor_copy(dst, src)
                    dma_eng = nc.sync if b == last_b else nc.gpsimd
                    dma_eng.dma_start(o_d[b][:, bk, :, :], dst)
                return f

            gi = 0
            for i, (b, kt) in enumerate(tiles):
                nkt = ns[b]
                if kt == 0:
                    state[b] = {
                        "pt": [],
                        "osb": osb_pool.tile([128, 2, 4, D + 1], F32,
                                             tag="osb", name=f"osb{b}"),
                    }
                sp = sp_pool.tile([128, L], F32, tag="sp")
                for h in range(2):
                    sl = slice(h * 512, (h + 1) * 512)
                    nc.tensor.matmul(sp[:, sl], ktm_sl(b, kt),
                                     qt_sl(b)[:, sl], start=True, stop=True)
                pt = pt_pool.tile([128, L], I16, tag="pt")
                state[b]["pt"].append(pt)
                emit_exp(b, kt, sp, pt, split=(i == 0))
                if kt == nkt - 1:
                    # O work becomes eligible 3 tiles later: by then the
                    # last exp of b has completed (sp recycling implies exp
                    # lag < 3), so the in-order PE queue never stalls on it.
                    owork.append((i + 3, o_unit_bank(b, 0)))
                    owork.append((i + 3, o_unit_bank(b, 1)))
                while owork and owork[0][0] <= i:
                    owork.popleft()[1]()
            while owork:
                owork.popleft()[1]()

    nc.compile()
    return nc


def get_program(ns):
    ns = tuple(ns)
    if ns not in _prog_cache:
        _prog_cache[ns] = _build_program(ns)
    return _prog_cache[ns]


def _prep_inputs(q, k, v, vl):
    """q,k,v: [n, L, D] f32; vl: [n] int. Returns (qkt, vp) bf16 arrays."""
    n = q.shape[0]
    qkt = np.zeros((n, D, 2 * L + 128), BF16NP)
    qt = (q.transpose(0, 2, 1) * np.float32(1.0 / np.sqrt(D))).astype(BF16NP)
    zmask = vl == 0
    if zmask.any():
        qt[zmask] = 0
    ktm = k.transpose(0, 2, 1).astype(BF16NP)
    qkt[:, :, :128] = ktm[:, :, :128]
    qkt[:, :, 128:128 + L] = qt
    qkt[:, :, 128 + L:2 * L] = ktm[:, :, 128:]
    vp = np.empty((n, L, D + 1), np.float32)
    vp[:, :, :D] = v
    vp[:, :, D] = 1.0
    iota = np.arange(L)
    dead = (iota[None, :] >= vl[:, None]) & ~zmask[:, None]
    vp[dead] = 0.0
    vp = vp.astype(BF16NP)
    vp = np.ascontiguousarray(
        vp.reshape(n, KT, 128, D + 1).transpose(0, 2, 1, 3))
    return qkt, vp


def kernel(queries, keys, values, valid_lens):
    queries = np.asarray(queries, np.float32)
    keys = np.asarray(keys, np.float32)
    values = np.asarray(values, np.float32)
    vl = np.asarray(valid_lens).astype(np.int64)

    # Ragged load balancing: sort batches by active k-tile count descending,
    # deal across cores; slot s runs max-of-group tiles on every core.
    nact = np.where(vl == 0, KT, -(-vl // 128)).astype(np.int64)
    order = np.argsort(-nact, kind="stable")
    ns = tuple(int(nact[order[s * N_CORES]]) for s in range(BPC))

    qkt, vp = _prep_inputs(queries[order], keys[order], values[order],
                           vl[order])

    nc = get_program(ns)
    in_maps = []
    for c in range(N_CORES):
        idx = [s * N_CORES + c for s in range(BPC)]
        in_maps.append({
            "qkt": np.ascontiguousarray(qkt[idx]),
            "vp": np.ascontiguousarray(vp[idx]),
        })

    res = None
    for attempt in range(3):
        try:
            res = run_bass_kernel_spmd(nc, in_maps, list(range(N_CORES)))
            break
        except Exception:
            if attempt == 2:
                raise
            import time as _time
            _time.sleep(2.0)
            try:
                import jax
                jax.clear_caches()
            except Exception:
                pass

    out = np.empty((B, L, D), np.float32)
    for c in range(N_CORES):
        o = res.results[c]["o"]  # [BPC, 128, 2, 4, D+1]
        o = np.asarray(o, np.float32).reshape(BPC, 128, KT, D + 1)
        o = o.transpose(0, 2, 1, 3).reshape(BPC, L, D + 1)
        on = o[:, :, :D] / o[:, :, D:D + 1]
        for s in range(BPC):
            out[order[s * N_CORES + c]] = on[s]
    return out
